# revision 25
# baseline (speedup 1.0000x reference)
"""Trainium2 Bass kernel for nn_Block_9345848836513.

Per-core pipeline (8 cores = 4 batches x 2 channel-halves, 16 ch each):
  1. channel mix in fp16 on PE: lhsT = x-chunk [128 rows = 4 consecutive
     128-tau windows x 32 ch, 128 taus], rhs = block-diag 4x mixer
     [128, 64] -> psum [tau, (window, ch)]; 256 matmuls, full 128-row
     contraction, one psum tag.  x arrives from HBM as fp16 (halves the
     phase-A DMA, its critical path).  Copies scatter psum into BigX
     laid out [p, ch, j-parity, frame] (fp16) so later folds read
     contiguous rows.
  2. forward rfft of 512-sample frames (hop 256) via radix-4-folded real
     DFT: DVE folds produce C1/C2 (even-bin sources, 128 long) and
     B0/B1 (odd-bin halves); 6 fp16 matmuls per channel give all 512
     real DOFs (vs 16 for a dense [512,512]).
  3. per-frame recurrence out_i = (spec_i + out_{i-1}) * transfer via
     tensor_tensor_scan along the frame axis (fp32 state, fp16 out)
  4. inverse rfft with Hann folded into the fp16 matrix; overlap-add
     folded into PSUM accumulation (second matmul group reads with a
     one-column shift); tanh straight from PSUM.
BigX is double-buffered across reps so rep k+1's DMA/mix overlaps rep
k's transform phase; psum budget 2 (mix) + 3 (fwd) + 3 (inv) = 8 banks.
Output DMAs ride the Pool SWDGE queue, x loads the SP HWDGE queue.
"""

import numpy as np

import concourse.bass as bass
import concourse.mybir as mybir
import concourse.tile as tile
from concourse import bacc
from concourse.bass_utils import run_bass_kernel_spmd

WINDOW = 512
HOP = 256
NCOEF = 257
NDOF = 512
B, C, T = 4, 32, 131072
F = T // HOP          # 512 frames
CPC = 16              # channels per core
NCORES = 8
JCOLS = T // 128      # 1024 output columns per channel
FPAD = F + 1          # 513 frame slots per (ch, parity); last is zero pad
FP32 = mybir.dt.float32
FP32R = mybir.dt.float32r
FP16 = mybir.dt.float16
U16 = mybir.dt.uint16


RADIX4 = True    # radix-4 even-bin forward (6 mm/ch) vs radix-2 (8 mm/ch)


def _build_dft_matrices():
    w = np.arange(WINDOW)
    k = np.arange(NCOEF)
    ang = 2.0 * np.pi * np.outer(w, k) / WINDOW
    cos, sin = np.cos(ang), np.sin(ang)
    fmat = np.zeros((WINDOW, NDOF), np.float64)
    fmat[:, :NCOEF] = cos
    fmat[:, NCOEF:] = -sin[:, 1:256]
    hann = 0.5 - 0.5 * np.cos(2.0 * np.pi * w / WINDOW)
    g = np.zeros((NDOF, WINDOW), np.float64)
    g[0, :] = 1.0
    g[256, :] = cos[:, 256]
    for kk in range(1, 256):
        g[kk, :] = 2.0 * cos[:, kk]
        g[256 + kk, :] = -2.0 * sin[:, kk]
    g *= hann[None, :] / WINDOW

    # dof indexing in the plain layout: Re k -> k (0..256), Im k -> 256+k
    def dofs_re(ks):
        return list(ks)

    def dofs_im(ks):
        return [256 + kk for kk in ks if 1 <= kk <= 255]

    # Chunks of 128 dofs in bin-class order.  radix-4: classes k mod 4 =
    # 0/2/1/3 with fold sources C1 = x0+x1+x2+x3, C2 = x0-x1+x2-x3 (via
    # e-halves eh0 = x0+x2, eh1 = x1+x3), B0 = x0-x2, B1 = x1-x3; for
    # k=0 mod 4: X[k] = sum_n C1[n] W^nk; k=2 mod 4: C2; odd k:
    # X[k] = sum_n B0[n] W^nk + B1[n] W^(n+128)k.  radix-2: classes
    # even/odd with sources E0 = eh0' = x0+x2, E1 = x1+x3 (e-halves) and
    # B0/B1; even k: X[k] = sum E0 W^nk + E1 W^(n+128)k.
    # All coefficients are rows of the plain fmat.
    if RADIX4:
        chunk_k = [
            list(range(0, 257, 4)),
            list(range(2, 256, 4)),
            list(range(1, 256, 4)),
            list(range(3, 256, 4)),
        ]
    else:
        chunk_k = [
            list(range(0, 257, 2))[:65] + [],   # placeholder, rebuilt below
        ]
        # even bins: 129 Re + 127 Im = 256 dofs -> 2 chunks; odd: 2 chunks
        ev = dofs_re(range(0, 257, 2)) + dofs_im(range(0, 257, 2))
        od = dofs_re(range(1, 256, 2)) + dofs_im(range(1, 256, 2))
        assert len(ev) == 256 and len(od) == 256
    newperm = []
    if RADIX4:
        for ks in chunk_k:
            newperm += dofs_re(ks) + dofs_im(ks)
    else:
        newperm = ev + od
    newperm = np.asarray(newperm)
    assert newperm.size == NDOF and np.unique(newperm).size == NDOF

    p0, p1, p2, p3 = (newperm[128 * i:128 * (i + 1)] for i in range(4))
    if RADIX4:
        # 6 lhsT blocks: (C1->c0, C2->c1, B0->c2, B1->c2, B0->c3, B1->c3)
        f_blocks = np.stack([
            fmat[0:128, p0],
            fmat[0:128, p1],
            fmat[0:128, p2],
            fmat[128:256, p2],
            fmat[0:128, p3],
            fmat[128:256, p3],
        ], axis=1)                                    # [128, 6, 128]
    else:
        # 8 blocks: (E0->c0, E1->c0, E0->c1, E1->c1, B0->c2, B1->c2,
        #            B0->c3, B1->c3)
        f_blocks = np.stack([
            fmat[0:128, p0],
            fmat[128:256, p0],
            fmat[0:128, p1],
            fmat[128:256, p1],
            fmat[0:128, p2],
            fmat[128:256, p2],
            fmat[0:128, p3],
            fmat[128:256, p3],
        ], axis=1)                                    # [128, 8, 128]
    g2 = g[newperm, :]                                # [512, 512]
    g_l = g2.reshape(4, 128, 4, 128).transpose(1, 0, 2, 3)
    return (np.ascontiguousarray(f_blocks, np.float16),
            np.ascontiguousarray(g_l, np.float16), newperm)


def _build_program(detect_races=True, reps=1, lag=0, psp_bufs=3, pwp_bufs=3, pool_folds=False, nu=12):
    nc = bacc.Bacc("TRN2", target_bir_lowering=False, num_devices=NCORES,
                   detect_race_conditions=detect_races)
    # xq[32*b + c, G*128 + p] = x[c, G*512 + b*128 + p]  (fp16, host layout)
    xq = nc.dram_tensor("xq", [128, T // 4], FP16, kind="ExternalInput")
    # mix4[32*b + c, 16*b + d] = mixer[c, d0+d]; zero elsewhere (fp16)
    mix4 = nc.dram_tensor("mix4", [128, 4 * CPC], FP16, kind="ExternalInput")
    nblk = 6 if RADIX4 else 8
    fmat = nc.dram_tensor("fmat", [128, nblk, 128], FP16, kind="ExternalInput")
    gmat = nc.dram_tensor("gmat", [128, 4, 4, 128], FP16, kind="ExternalInput")
    trt = nc.dram_tensor("trt", [128, CPC * 4], FP32, kind="ExternalInput")
    gainv = nc.dram_tensor("gainv", [1, CPC], FP32, kind="ExternalInput")
    out_d = nc.dram_tensor("out", [CPC, 128, JCOLS], FP32, kind="ExternalOutput")

    ADD, MUL = mybir.AluOpType.add, mybir.AluOpType.mult
    SUB = mybir.AluOpType.subtract

    with tile.TileContext(nc) as tc:
        with (
            tc.tile_pool(name="singles", bufs=1) as singles,
            tc.tile_pool(name="xa", bufs=3) as xa,
            tc.tile_pool(name="pmix", bufs=2, space="PSUM") as pmix,
            tc.tile_pool(name="eo", bufs=3) as eop,
            tc.tile_pool(name="rp", bufs=3) as rp,
            tc.tile_pool(name="psp", bufs=psp_bufs, space="PSUM") as psp,
            tc.tile_pool(name="pwp", bufs=pwp_bufs, space="PSUM") as pwp,
        ):
            fsb = singles.tile([128, nblk, 128], FP16)
            gsb = singles.tile([128, 4, 4, 128], FP16)
            mix_sb = singles.tile([128, 4 * CPC], FP16)
            trsb = singles.tile([128, CPC * 4], FP32)
            gain_sb = singles.tile([128, CPC], FP32)
            # bigx[p, d, h, f] = y[d, t = 256f + 128h + p]  (j = 2f+h)
            bigxs = [singles.tile([128, CPC, 2, FPAD], FP16, name=f"bigx{r}")
                     for r in range(2)]
            NU = nu
            u_all = [singles.tile([128, 513], FP16, name=f"uall{i}")
                     for i in range(NU)]
            for ut in u_all:
                nc.vector.memset(ut[:, 0:1].bitcast(U16), 0)
            # params ride the SWDGE queue so the first phase-A x-load
            # (HWDGE) isn't queued behind the parameter DMAs
            nc.sync.dma_start(out=mix_sb[:], in_=mix4[:])
            nc.gpsimd.dma_start(out=fsb[:], in_=fmat[:])
            nc.gpsimd.dma_start(out=gsb[:], in_=gmat[:])
            nc.gpsimd.dma_start(out=trsb[:], in_=trt[:])
            nc.gpsimd.dma_start(out=gain_sb[:], in_=gainv[:].to_broadcast((128, CPC)))
            for bx in bigxs:
                nc.vector.memset(bx[:, :, :, F:FPAD].bitcast(U16), 0)

            def emit_a_tile(bigx, i):
                # ---- Phase A: transposed mix into BigX (fp16) ----
                # xt tile i holds groups G in [32i, 32(i+1)); matmul for
                # group G: lhsT = xt[:, local window] -> psum cols
                # [64*G' + 16*b + d] with j = 4G + b, h = b%2,
                # f = 2*(8s + G') + b//2  (s = psum slab index 4i+g).
                xt = xa.tile([128, 4096], FP16, tag="xt")
                nc.sync.dma_start(
                    out=xt[:], in_=xq[:, 4096 * i:4096 * (i + 1)])
                for g in range(4):
                    ps = pmix.tile([128, 512], FP32, tag="pmix")
                    for gp in range(8):
                        nc.tensor.matmul(
                            ps[:, 64 * gp:64 * (gp + 1)],
                            lhsT=xt[:, 1024 * g + 128 * gp:
                                    1024 * g + 128 * (gp + 1)],
                            rhs=mix_sb[:],
                        )
                    # psum col = G'*64 + b*16 + d, with b = 2*bf + h
                    # and frame offset fr = 2*G' + bf, i.e.
                    # col = 32*fr + 16*h + d -> "(fr h d)" grouping.
                    f0 = 16 * (4 * i + g)
                    nc.scalar.copy(
                        bigx[:, :, :, f0:f0 + 16],
                        ps[:].rearrange("p (fr h d) -> p d h fr",
                                        fr=16, h=2),
                    )

            for _rep in range(reps):
                bigx = bigxs[_rep % 2]
                if _rep == 0:
                    for i in range(8):
                        emit_a_tile(bigx, i)
                nxt = bigxs[(_rep + 1) % 2]

                # ---- Phase B: folds -> DFT -> scan -> inverse+OLA -> tanh
                # The next rep's phase A (DMA + mix + copies into the other
                # BigX buffer) is interleaved one i-tile per two channels so
                # the in-order engine queues never see a rep-boundary stall.
                LAG = lag
                us_by_d = {}
                for dd in range(CPC + LAG):
                  if dd < CPC and dd % 2 == 1 and _rep + 1 < reps:
                    emit_a_tile(nxt, dd // 2)
                  if dd < CPC:
                    d = dd
                    bx0 = bigx[:, d, 0, :]
                    bx1 = bigx[:, d, 1, :]
                    eh0 = eop.tile([128, 512], FP16, tag="eh0")
                    eh1 = eop.tile([128, 512], FP16, tag="eh1")
                    if RADIX4:
                        c1t = eop.tile([128, 512], FP16, tag="c1")
                        c2t = eop.tile([128, 512], FP16, tag="c2")
                    b0t = eop.tile([128, 512], FP16, tag="b0")
                    b1t = eop.tile([128, 512], FP16, tag="b1")
                    nc.vector.tensor_tensor(eh0[:], bx0[:, 0:512], bx0[:, 1:513], op=ADD)
                    nc.vector.tensor_tensor(eh1[:], bx1[:, 0:512], bx1[:, 1:513], op=ADD)
                    if RADIX4:
                        nc.vector.tensor_tensor(c1t[:], eh0[:], eh1[:], op=ADD)
                        nc.vector.tensor_tensor(c2t[:], eh0[:], eh1[:], op=SUB)
                    fold_eng = nc.gpsimd if pool_folds else nc.vector
                    fold_eng.tensor_tensor(b0t[:], bx0[:, 0:512], bx0[:, 1:513], op=SUB)
                    fold_eng.tensor_tensor(b1t[:], bx1[:, 0:512], bx1[:, 1:513], op=SUB)
                    # (lhsT block, rhs tile) per m-chunk
                    if RADIX4:
                        plan = [
                            [(0, c1t)],
                            [(1, c2t)],
                            [(2, b0t), (3, b1t)],
                            [(4, b0t), (5, b1t)],
                        ]
                    else:
                        plan = [
                            [(0, eh0), (1, eh1)],
                            [(2, eh0), (3, eh1)],
                            [(4, b0t), (5, b1t)],
                            [(6, b0t), (7, b1t)],
                        ]
                    us = []
                    for m in range(4):
                        ps = psp.tile([128, 512], FP32)
                        terms = plan[m]
                        for ti, (blk, src) in enumerate(terms):
                            nc.tensor.matmul(
                                ps[:],
                                lhsT=fsb[:, blk, :],
                                rhs=src[:],
                                start=(ti == 0),
                                stop=(ti == len(terms) - 1),
                            )
                        # col 0 zero pad keeps the shifted (f-1) OLA read
                        # 512 wide with a well-formed psum group
                        u = u_all[(d * 4 + m) % NU]
                        idx = d * 4 + m
                        nc.vector.tensor_tensor_scan(
                            u[:, 1:513], ps[:],
                            trsb[:, idx:idx + 1].broadcast_to((128, 512)),
                            0.0, op0=ADD, op1=MUL,
                        )
                        us.append(u)
                    us_by_d[d] = us
                  if dd >= LAG:
                    d = dd - LAG
                    us = us_by_d.pop(d)
                    # inverse DFT with overlap-add folded into PSUM:
                    # out col j=2f+s gets W_s[:,f] + W_{s+2}[:,f-1]
                    res = rp.tile([128, JCOLS], FP32)
                    ov = res[:].rearrange("p (f two) -> p two f", two=2)
                    for s01 in range(2):
                        pout = pwp.tile([128, 512], FP32)
                        for k in range(4):
                            nc.tensor.matmul(
                                pout[:],
                                lhsT=gsb[:, k, s01, :],
                                rhs=us[k][:, 1:513],
                                start=(k == 0),
                                stop=False,
                            )
                        for k in range(4):
                            nc.tensor.matmul(
                                pout[:],
                                lhsT=gsb[:, k, s01 + 2, :],
                                rhs=us[k][:, 0:512],
                                start=False,
                                stop=(k == 3),
                            )
                        nc.scalar.activation(
                            ov[:, s01, :], pout[:],
                            mybir.ActivationFunctionType.Tanh,
                            scale=gain_sb[:, d:d + 1],
                        )
                    nc.gpsimd.dma_start(out=out_d[d], in_=res[:])
    nc.compile()
    return nc


def build_in_maps(x, transfer, mixer_matrix, gain):
    f_blocks, g_l, newperm = _build_dft_matrices()

    # transfer per dof (re/im parts share the same real coefficient),
    # permuted into the chunked dof order
    tr_plain = np.empty((C, NDOF), np.float32)
    tr_plain[:, :NCOEF] = transfer
    tr_plain[:, NCOEF:] = transfer[:, 1:256]
    tr_dof = np.ascontiguousarray(tr_plain[:, newperm])

    in_maps = []
    for core in range(NCORES):
        b, h = core // 2, core % 2
        d0 = h * CPC
        mixcols = mixer_matrix[:, d0:d0 + CPC]               # [32, 16]
        mix4 = np.zeros((128, 4 * CPC), np.float16)
        for q in range(4):
            mix4[32 * q:32 * (q + 1), CPC * q:CPC * (q + 1)] = mixcols
        trd = tr_dof[d0:d0 + CPC]                            # [16, 512]
        trt = np.ascontiguousarray(
            trd.reshape(CPC, 4, 128).transpose(2, 0, 1).reshape(128, CPC * 4))
        # xq[32*bq + c, G*128 + p] = x[c, G*512 + bq*128 + p]
        xqv = np.ascontiguousarray(
            x[b].reshape(C, T // 512, 4, 128).transpose(2, 0, 1, 3)
            .reshape(128, T // 4).astype(np.float16))
        in_maps.append({
            "xq": xqv,
            "mix4": mix4,
            "fmat": f_blocks,
            "gmat": g_l,
            "trt": trt,
            "gainv": np.ascontiguousarray(gain[d0:d0 + CPC].reshape(1, CPC)),
        })
    return in_maps


_PROGRAM_CACHE = {}


def kernel(x, transfer, mixer_matrix, gain, **run_kwargs):
    x = np.ascontiguousarray(x, np.float32)
    transfer = np.asarray(transfer, np.float32)
    mixer_matrix = np.asarray(mixer_matrix, np.float32)
    gain = np.asarray(gain, np.float32)

    in_maps = build_in_maps(x, transfer, mixer_matrix, gain)

    if "nc" not in _PROGRAM_CACHE:
        _PROGRAM_CACHE["nc"] = _build_program()
    nc = _PROGRAM_CACHE["nc"]

    res = run_bass_kernel_spmd(nc, in_maps, list(range(NCORES)), **run_kwargs)

    out = np.empty((B, C, T), np.float32)
    for core in range(NCORES):
        b, h = core // 2, core % 2
        o = res.results[core]["out"]                    # [16, 128, 1024]
        out[b, h * CPC:(h + 1) * CPC] = o.transpose(0, 2, 1).reshape(CPC, T)
    kernel.last_results = res
    return out
